# revision 9
# baseline (speedup 1.0000x reference)
"""Bass/Trainium2 kernel v5 for nn_BmmEnsemble (ensemble-of-MLPs energy sum).

Sharding: 8 cores; core c owns species c//2, half c%2 (6250 atoms).

v5 design (from v4 trace analysis: Tensor/DVE/ACT all ~88% busy):
  - All three matmul layers run fp8 DoubleRow: L1 8 instrs (as v4), L2
    1 DR (K=256) + K=2 bf16 hi/lo bias matmul per output group, L3 1 DR
    (K=194+3 bias rows in the kc1 slack) per output group.
  - Layer scales S1=8, S2=16, S3=128 (power-of-2, absorbed host-side).
  - Bias lives IN z everywhere, so each tile's CELU is ONE exp (ACT,
    const bias) + ONE stt (DVE): g = (u min S*alpha) max z, writing fp8
    for the next layer's DR moving operand ([128, 2, CH], kc-blocked).
  - fp8 W2/W3 systematic quant error is mean-corrected via sampled
    E[g1], E[g2] folded into the (exact) bias rows at prep time.
  - L2's 64-feature B-half pairs 2 slots into one PSUM bank (partition
    offsets 0/64) -> exp once per pair; L3's 32-feature tail packs 4
    slots (offsets 0/32/64/96). w3 stationaries are parity-permuted so
    odd slots' kc1 features sit at partitions 64-127.
  - L3 sums ride the stt accum_out (fp32, pre-quantization); L4 + mean
    + final sum in fp64 on host.
"""

import numpy as np
import ml_dtypes

import concourse.bacc as bacc
import concourse.tile as tile
import concourse.mybir as mybir
from concourse.bass_utils import run_bass_kernel_spmd

F32 = mybir.dt.float32
BF16 = mybir.dt.bfloat16
F8 = mybir.dt.float8e4
DR = mybir.MatmulPerfMode.DoubleRow
AF = mybir.ActivationFunctionType
ALU = mybir.AluOpType

NP_F8 = ml_dtypes.float8_e4m3
NP_BF = ml_dtypes.bfloat16

S = 4
E = 8
N = 50000
AEV = 1008
ALPHA = 0.1
NCORES = 8
NA = N // S // 2           # atoms per core: 6250
CH = 512                   # atom chunk (matmul free dim)
NCH = (NA + CH - 1) // CH  # 13 chunks (12 x 512 + 106)
NSLOT = E * NCH            # 104 pipeline slots
NPK = NSLOT // 4           # 26 groups of 4 slots sharing the zv bank
S1 = 8.0
S2 = 16.0
S3 = 128.0
D1, D2, D3 = 256, 192, 160
NCORR = 8192               # atoms sampled for mean-correction

# exp consts: u = exp(z/(Ssc*a) + ln(Ssc*a) - 1) = Ssc*a*e^{y/a}
#   (z = Ssc*(y+a) includes bias)
EB1 = float(np.log(S1 * ALPHA) - 1.0)
EB2 = float(np.log(S2 * ALPHA) - 1.0)
EB3 = float(np.log(S3 * ALPHA) - 1.0)


def _build(dbg=False):
    nc = bacc.Bacc("TRN2", target_bir_lowering=False, debug=False,
                   num_devices=NCORES)

    x8 = nc.dram_tensor("x8", [4, 128, 2, NA], F8, kind="ExternalInput")
    w1 = nc.dram_tensor("w1", [E, 4, 128, 2, D1], F8, kind="ExternalInput")
    w2 = nc.dram_tensor("w2", [E, 128, 2, 256], F8, kind="ExternalInput")
    w2b = nc.dram_tensor("w2b", [E, 2, 128], BF16, kind="ExternalInput")
    w2bp = nc.dram_tensor("w2bp", [E // 2, 2, 128], BF16, kind="ExternalInput")
    w3 = nc.dram_tensor("w3", [E, 128, 2, 256], F8, kind="ExternalInput")
    g2i = nc.dram_tensor("g2i", [64, CH], F8, kind="ExternalInput")
    accA = nc.dram_tensor("accA", [128, NSLOT], F32, kind="ExternalOutput")
    accB = nc.dram_tensor("accB", [128, NPK], F32, kind="ExternalOutput")

    with tile.TileContext(nc) as tc:
        with (
            tc.tile_pool(name="wp", bufs=1) as wp,
            tc.tile_pool(name="xp", bufs=2) as xp,
            tc.tile_pool(name="g1p", bufs=4) as g1p,
            tc.tile_pool(name="g2p", bufs=6) as g2p,
            tc.tile_pool(name="up", bufs=3) as up,
            tc.tile_pool(name="sp", bufs=2) as sp,
            tc.tile_pool(name="ps", bufs=5, space="PSUM") as ps,
            tc.tile_pool(name="psb", bufs=2, space="PSUM") as psb,
            tc.tile_pool(name="psv", bufs=1, space="PSUM") as psv,
        ):
            # ---- x prefetch ----
            xtiles = {}

            def emit_x(ci):
                if ci >= NCH or ci in xtiles:
                    return
                off = ci * CH
                na = min(CH, NA - off)
                lst = []
                for p in range(4):
                    t = xp.tile([128, 2, CH], F8, tag=f"x{p}")
                    nc.sync.dma_start(t[:, :, :na], x8[p, :, :, off:off + na])
                    lst.append(t)
                xtiles[ci] = lst

            emit_x(0)
            emit_x(1)

            # ---- resident weights (e-major so e=0 lands first) ----
            w1t, w2t, w2bt, w2bpt, w3t = {}, {}, {}, {}, {}
            for e in range(E):
                for p in range(4):
                    t = wp.tile([128, 2, D1], F8, tag=f"w1_{e}_{p}")
                    nc.sync.dma_start(t[:], w1[e, p])
                    w1t[e, p] = t
                t = wp.tile([128, 2, 256], F8, tag=f"w2_{e}")
                nc.sync.dma_start(t[:], w2[e])
                w2t[e] = t
                t = wp.tile([2, 128], BF16, tag=f"w2b_{e}")
                nc.sync.dma_start(t[:], w2b[e])
                w2bt[e] = t
                if e % 2 == 0:
                    t = wp.tile([2, 128], BF16, tag=f"w2bp_{e // 2}")
                    nc.sync.dma_start(t[:], w2bp[e // 2])
                    w2bpt[e // 2] = t
                t = wp.tile([128, 2, 256], F8, tag=f"w3_{e}")
                nc.sync.dma_start(t[:], w3[e])
                w3t[e] = t
            accAt = wp.tile([128, NSLOT], F32, tag="accA")
            accBt = wp.tile([128, NPK], F32, tag="accB")
            ones2 = wp.tile([2, CH], BF16, tag="ones2")
            nc.vector.memset(ones2[:], 1.0)
            eb1c = wp.tile([128, 1], F32, tag="eb1c")
            nc.vector.memset(eb1c[:], EB1)
            eb2c = wp.tile([128, 1], F32, tag="eb2c")
            nc.vector.memset(eb2c[:], EB2)
            eb3c = wp.tile([128, 1], F32, tag="eb3c")
            nc.vector.memset(eb3c[:], EB3)

            state = {}
            pstate = {}   # zB pair banks, keyed it//2
            vstate = {}   # zv pack banks, keyed it//4

            def slot_info(it):
                ci, e = divmod(it, E)
                na = min(CH, NA - ci * CH)
                return ci, e, na

            def l1_mm(it):
                ci, e, na = slot_info(it)
                if it % E == 0:
                    emit_x(ci + 1)
                st = state.setdefault(it, {})
                st["z1"] = []
                for mi in range(2):
                    z = ps.tile([128, CH], F32, tag="z")
                    for p in range(4):
                        nc.tensor.matmul(
                            z[:, :na],
                            w1t[e, p][:, :, mi * 128:(mi + 1) * 128],
                            xtiles[ci][p][:, :, :na],
                            start=(p == 0), stop=(p == 3), perf_mode=DR,
                        )
                    st["z1"].append(z)

            def l1_ew(it):
                ci, e, na = slot_info(it)
                st = state[it]
                g1 = g1p.tile([128, 2, CH], F8, tag="g1")
                for mi in range(2):
                    z = st["z1"][mi]
                    u = up.tile([128, CH], BF16, tag=f"u1{mi}")
                    nc.scalar.activation(u[:, :na], z[:, :na], AF.Exp,
                                         bias=eb1c[:, 0:1], scale=float(10.0 / S1))
                    nc.vector.scalar_tensor_tensor(
                        g1[:, mi, :na], u[:, :na], float(S1 * ALPHA),
                        z[:, :na], op0=ALU.min, op1=ALU.max)
                st["g1"] = g1
                del st["z1"]

            def l2_mm(it):
                ci, e, na = slot_info(it)
                st = state[it]
                g1 = st["g1"]
                zA = ps.tile([128, CH], F32, tag="z")
                nc.tensor.matmul(zA[:, :na], w2bt[e][:, 0:128],
                                 ones2[:, :na], start=True, stop=False)
                nc.tensor.matmul(zA[:, :na], w2t[e][:, :, 0:128],
                                 g1[:, :, :na], start=False, stop=True,
                                 perf_mode=DR)
                k = it % 2
                if k == 0:
                    zb = psb.tile([128, CH], F32, tag="zbp", name="zbp")
                    pstate[it // 2] = zb
                    nc.tensor.matmul(zb[:, :na], w2bpt[e // 2][:, 0:128],
                                     ones2[:, :na], start=True, stop=False)
                zb = pstate[it // 2]
                if k == 0:
                    nc.tensor.matmul(zb[0:64, :na], w2t[e][:, :, 128:192],
                                     g1[:, :, :na], start=False, stop=False,
                                     perf_mode=DR)
                else:
                    # odd e: B features at weight cols 192-255 -> partitions
                    # 64-127; cols 128-191 are zero (accumulate +0 on even's)
                    nc.tensor.matmul(zb[:, :na], w2t[e][:, :, 128:256],
                                     g1[:, :, :na], start=False, stop=True,
                                     perf_mode=DR)
                st["zA"] = zA

            def l2_ew(it):
                ci, e, na = slot_info(it)
                st = state[it]
                zA = st["zA"]
                g2 = g2p.tile([128, 2, CH], F8, tag="g2")
                if it < 6:
                    # one-time per pool buf (bufs=4, parity-stable):
                    # bias const rows (1, 1/16, 1/256) + zeroed junk
                    # partitions for the L3 DR contraction (NaN*0 = NaN)
                    if it % 2 == 0:
                        nc.sync.dma_start(g2[64:128, 1, :], g2i[:])
                    else:
                        nc.sync.dma_start(g2[0:64, 1, :], g2i[:])
                uA = up.tile([128, CH], BF16, tag="u2A")
                nc.scalar.activation(uA[:, :na], zA[:, :na], AF.Exp,
                                     bias=eb2c[:, 0:1], scale=float(10.0 / S2))
                nc.vector.scalar_tensor_tensor(
                    g2[:, 0, :na], uA[:, :na], float(S2 * ALPHA),
                    zA[:, :na], op0=ALU.min, op1=ALU.max)
                st["g2"] = g2
                if it % 2 == 1:
                    zb = pstate.pop(it // 2)
                    uB = up.tile([128, CH], BF16, tag="u2B")
                    nc.scalar.activation(uB[:, :na], zb[:, :na], AF.Exp,
                                         bias=eb2c[:, 0:1], scale=float(10.0 / S2))
                    g2ev = state[it - 1]["g2"]
                    nc.vector.scalar_tensor_tensor(
                        g2ev[0:64, 1, :na], uB[0:64, :na], float(S2 * ALPHA),
                        zb[0:64, :na], op0=ALU.min, op1=ALU.max)
                    nc.vector.scalar_tensor_tensor(
                        g2[64:128, 1, :na], uB[64:128, :na], float(S2 * ALPHA),
                        zb[64:128, :na], op0=ALU.min, op1=ALU.max)

            def l3_mm(it):
                ci, e, na = slot_info(it)
                st = state[it]
                g2 = st["g2"]
                zA = ps.tile([128, CH], F32, tag="z")
                nc.tensor.matmul(zA[:, :na], w3t[e][:, :, 0:128],
                                 g2[:, :, :na], start=True, stop=True,
                                 perf_mode=DR)
                k = it % 4
                if k == 0:
                    vstate[it // 4] = psv.tile([128, CH], F32, tag="zvp",
                                               name="zvp")
                zv = vstate[it // 4]
                # tail features at weight cols 128+32k..128+32k+31; all other
                # cols zero -> full-M write at tile position (0,0)
                nc.tensor.matmul(zv[:, :na], w3t[e][:, :, 128:256],
                                 g2[:, :, :na], start=(k == 0), stop=(k == 3),
                                 perf_mode=DR)
                st["z3"] = zA

            def l3_ew(it):
                ci, e, na = slot_info(it)
                st = state[it]
                zA = st["z3"]
                u3 = up.tile([128, CH], BF16, tag="u3")
                nc.scalar.activation(u3[:, :na], zA[:, :na], AF.Exp,
                                     bias=eb3c[:, 0:1], scale=float(10.0 / S3))
                g3 = sp.tile([128, CH], BF16, tag="g3")
                nc.vector.scalar_tensor_tensor(
                    g3[:, :na], u3[:, :na], float(S3 * ALPHA), zA[:, :na],
                    op0=ALU.min, op1=ALU.max,
                    accum_out=accAt[:, it:it + 1])
                if it % 4 == 3:
                    zv = vstate.pop(it // 4)
                    uB = up.tile([128, CH], BF16, tag="u3B")
                    nc.scalar.activation(uB[:, :na], zv[:, :na], AF.Exp,
                                         bias=eb3c[:, 0:1], scale=float(10.0 / S3))
                    gB = sp.tile([128, CH], BF16, tag="g3B")
                    nc.vector.scalar_tensor_tensor(
                        gB[:, :na], uB[:, :na], float(S3 * ALPHA),
                        zv[:, :na], op0=ALU.min, op1=ALU.max,
                        accum_out=accBt[:, it // 4:it // 4 + 1])
                del state[it]

            # ---- software-pipelined main loop ----
            # stage offsets 2 apart so PE never waits on same-iteration
            # ACT/DVE g-tile writes; all matmuls emitted before elementwise
            for t in range(NSLOT + 4):
                if t < NSLOT:
                    l1_mm(t)
                if 2 <= t < NSLOT + 2:
                    l2_mm(t - 2)
                if t >= 4:
                    l3_mm(t - 4)
                if t < NSLOT:
                    l1_ew(t)
                if 2 <= t < NSLOT + 2:
                    l2_ew(t - 2)
                if t >= 4:
                    l3_ew(t - 4)

            nc.sync.dma_start(accA[:], accAt[:])
            nc.sync.dma_start(accB[:], accBt[:])
    nc.compile()
    return nc


_NC = None


def _get_nc():
    global _NC
    if _NC is None:
        _NC = _build()
    return _NC


def _f8(x):
    return np.clip(x, -240.0, 240.0).astype(NP_F8).astype(np.float32)


def _layer_host(z, Ssc):
    with np.errstate(over="ignore"):
        u = np.exp(z * np.float32(1.0 / (Ssc * ALPHA))
                   + np.float32(np.log(Ssc * ALPHA) - 1.0)
                   ).astype(NP_BF).astype(np.float32)
    return np.maximum(np.minimum(u, np.float32(Ssc * ALPHA)), z)


def _prep_inputs(inputs):
    aev = np.asarray(inputs["aev"], dtype=np.float32).reshape(N, AEV)
    idx = np.asarray(inputs["idx"])
    Ws = [np.asarray(inputs[f"W{i}"], dtype=np.float32) for i in (1, 2, 3, 4)]
    bs = [np.asarray(inputs[f"b{i}"], dtype=np.float32) for i in (1, 2, 3, 4)]

    in_maps = []
    per_species = {}
    for c in range(NCORES):
        s, h = c // 2, c % 2
        if s not in per_species:
            # ---- L1 fp8 stack with dual bias rows (as v4) ----
            w1s = np.zeros((E, 1024, D1), dtype=np.float32)
            w1s[:, :AEV, :] = Ws[0][s] * S1
            bt = (bs[0][s][:, 0, :] + ALPHA) * S1           # [E, 256]
            r1 = _f8(bt)
            r2 = _f8((bt - r1) * 16.0)
            w1s[:, AEV, :] = r1
            w1s[:, AEV + 1, :] = r2
            w1q = w1s.astype(NP_F8)
            w1pk = np.ascontiguousarray(
                w1q.reshape(E, 4, 2, 128, D1).transpose(0, 1, 3, 2, 4))

            # ---- sample atoms for mean-correction ----
            sel_s = np.asarray(idx[s])
            samp = aev[sel_s[:NCORR]]
            xq_s = _f8(samp)
            xaug = np.concatenate(
                [xq_s, np.ones((xq_s.shape[0], 1), np.float32),
                 np.full((xq_s.shape[0], 1), 1.0 / 16.0, np.float32)], axis=1)

            # ---- L2 fp8 DR stack + bias hi/lo + mean corr ----
            w2q_f = np.empty((E, D1, D2), dtype=np.float32)
            pre2 = np.empty((E, D2), dtype=np.float64)
            w3q_f = np.empty((E, 192, D3), dtype=np.float32)
            b3t = np.empty((E, D3), dtype=np.float64)
            for e in range(E):
                w2q = _f8(Ws[1][s, e] * (S2 / S1))           # [256, 192]
                w2q_f[e] = w2q
                b2p = bs[1][s, e, 0].astype(np.float64) - ALPHA * (
                    w2q.astype(np.float64) * (S1 / S2)).sum(axis=0)
                p2 = S2 * (b2p + ALPHA)
                # mean correction via sampled g1
                w1full = w1q[e, :AEV + 2].astype(np.float32)
                z1s = xaug @ w1full           # bias rows included via xaug
                g1s = _f8(_layer_host(z1s, S1))
                dW2 = w2q.astype(np.float64) - Ws[1][s, e].astype(np.float64) * (S2 / S1)
                Eg1 = g1s.astype(np.float64).mean(axis=0)
                p2 = p2 - dW2.T @ Eg1
                pre2[e] = p2

                w3q = _f8(Ws[2][s, e] * (S3 / S2))           # [192, 160]
                w3q_f[e] = w3q
                b3p = bs[2][s, e, 0].astype(np.float64) - ALPHA * (
                    w3q.astype(np.float64) * (S2 / S3)).sum(axis=0)
                bt3 = S3 * (b3p + ALPHA)
                z2s = g1s @ w2q + p2.astype(np.float32)
                g2s = _f8(_layer_host(z2s, S2))
                dW3 = w3q.astype(np.float64) - Ws[2][s, e].astype(np.float64) * (S3 / S2)
                Eg2 = g2s.astype(np.float64).mean(axis=0)
                b3t[e] = bt3 - dW3.T @ Eg2

            # pack L2 stationary [E, 128, 2, 256]: k = kc*128 + part;
            # cols 0-127 = A feats; B feats at 128-191 (even e) / 192-255
            # (odd e), zero elsewhere (-> partition-packed pair writes)
            w2pad = np.zeros((E, D1, 256), dtype=np.float32)
            w2pad[:, :, 0:128] = w2q_f[:, :, 0:128]
            for e in range(E):
                if e % 2 == 0:
                    w2pad[e, :, 128:192] = w2q_f[e, :, 128:192]
                else:
                    w2pad[e, :, 192:256] = w2q_f[e, :, 128:192]
            w2pk = np.ascontiguousarray(
                w2pad.astype(NP_F8).reshape(E, 2, 128, 256).transpose(0, 2, 1, 3))
            # L2 bias hi/lo rows
            p2hi = pre2.astype(np.float32).astype(NP_BF)
            p2lo = (pre2 - p2hi.astype(np.float64)).astype(np.float32).astype(NP_BF)
            w2bpk = np.zeros((E, 2, 128), dtype=NP_BF)
            w2bpk[:, 0, :] = p2hi[:, :128]
            w2bpk[:, 1, :] = p2lo[:, :128]
            w2bppk = np.zeros((E // 2, 2, 128), dtype=NP_BF)
            for pe in range(E // 2):
                w2bppk[pe, 0, 0:64] = p2hi[2 * pe, 128:192]
                w2bppk[pe, 1, 0:64] = p2lo[2 * pe, 128:192]
                w2bppk[pe, 0, 64:128] = p2hi[2 * pe + 1, 128:192]
                w2bppk[pe, 1, 64:128] = p2lo[2 * pe + 1, 128:192]

            # ---- L3 fp8 DR stack, parity kc1 layout + 3 fp8 bias rows ----
            # M cols: 0-127 = features 0-127; 128-255 = zv block where slot
            # k=e%4 owns cols 128+32k..128+32k+31 (features 128-159)
            b1r = _f8(b3t)
            b2r = _f8((b3t - b1r) * 16.0)
            b3r = _f8((b3t - b1r - b2r / 16.0) * 256.0)
            w3pk = np.zeros((E, 128, 2, 256), dtype=NP_F8)
            for e in range(E):
                wfull = np.zeros((192, 256), dtype=np.float32)  # [g2feat, M]
                brow = np.zeros((3, 256), dtype=np.float32)
                wfull[:, 0:128] = w3q_f[e][:, 0:128]
                brow[0, 0:128] = b1r[e, 0:128]
                brow[1, 0:128] = b2r[e, 0:128]
                brow[2, 0:128] = b3r[e, 0:128]
                k = e % 4
                c0 = 128 + 32 * k
                wfull[:, c0:c0 + 32] = w3q_f[e][:, 128:160]
                brow[0, c0:c0 + 32] = b1r[e, 128:160]
                brow[1, c0:c0 + 32] = b2r[e, 128:160]
                brow[2, c0:c0 + 32] = b3r[e, 128:160]
                w3pk[e, :, 0, :] = wfull[0:128].astype(NP_F8)
                if e % 2 == 0:
                    w3pk[e, 0:64, 1, :] = wfull[128:192].astype(NP_F8)
                    w3pk[e, 64:67, 1, :] = brow.astype(NP_F8)
                else:
                    w3pk[e, 64:128, 1, :] = wfull[128:192].astype(NP_F8)
                    w3pk[e, 0:3, 1, :] = brow.astype(NP_F8)

            per_species[s] = (w1pk, w2pk, w2bpk, w2bppk, w3pk)

        w1pk, w2pk, w2bpk, w2bppk, w3pk = per_species[s]
        sel = np.asarray(idx[s, h * NA:(h + 1) * NA])
        xTc = np.zeros((1024, NA), dtype=np.float32)
        xTc[:AEV] = aev[sel].T
        xTc[AEV] = 1.0
        xTc[AEV + 1] = 1.0 / 16.0
        x8c = np.ascontiguousarray(
            xTc.astype(NP_F8).reshape(4, 2, 128, NA).transpose(0, 2, 1, 3))

        g2init = np.zeros((64, CH), dtype=NP_F8)
        g2init[0, :] = 1.0
        g2init[1, :] = 1.0 / 16.0
        g2init[2, :] = 1.0 / 256.0
        in_maps.append({
            "x8": x8c, "w1": w1pk, "w2": w2pk, "w2b": w2bpk,
            "w2bp": w2bppk, "w3": w3pk, "g2i": g2init,
        })
    return in_maps, Ws, bs


def _finish(results, Ws, bs):
    W4 = Ws[3].astype(np.float64)  # [S, E, 160, 1]
    b4 = bs[3].astype(np.float64)  # [S, E, 1, 1]
    total = 0.0
    for c in range(NCORES):
        s = c // 2
        aA = results[c]["accA"].astype(np.float64)  # [128, NSLOT]
        aB = results[c]["accB"].astype(np.float64)  # [128, NPK]
        for e in range(E):
            g3sum = np.zeros(D3, dtype=np.float64)
            for ci in range(NCH):
                it = ci * E + e
                g3sum[:128] += aA[:, it]
                k = it % 4
                g3sum[128:160] += aB[32 * k:32 * k + 32, it // 4]
            h3sum = g3sum / S3 - ALPHA * NA
            total += (h3sum @ W4[s, e, :, 0] + NA * b4[s, e, 0, 0]) / E
    return np.array([total], dtype=np.float32)


def _run(inputs, **spmd_kwargs):
    in_maps, Ws, bs = _prep_inputs(inputs)
    nc = _get_nc()
    res = run_bass_kernel_spmd(nc, in_maps, list(range(NCORES)), **spmd_kwargs)
    return _finish(res.results, Ws, bs), res


def kernel(**inputs) -> np.ndarray:
    out, _ = _run(inputs)
    return out


# revision 10
# speedup vs baseline: 1.0250x; 1.0250x over previous
"""Bass/Trainium2 kernel v5 for nn_BmmEnsemble (ensemble-of-MLPs energy sum).

Sharding: 8 cores; core c owns species c//2, half c%2 (6250 atoms).

v5 design (from v4 trace analysis: Tensor/DVE/ACT all ~88% busy):
  - All three matmul layers run fp8 DoubleRow: L1 8 instrs (as v4), L2
    1 DR (K=256) + K=2 bf16 hi/lo bias matmul per output group, L3 1 DR
    (K=194+3 bias rows in the kc1 slack) per output group.
  - Layer scales S1=8, S2=16, S3=128 (power-of-2, absorbed host-side).
  - Bias lives IN z everywhere, so each tile's CELU is ONE exp (ACT,
    const bias) + ONE stt (DVE): g = (u min S*alpha) max z, writing fp8
    for the next layer's DR moving operand ([128, 2, CH], kc-blocked).
  - fp8 W2/W3 systematic quant error is mean-corrected via sampled
    E[g1], E[g2] folded into the (exact) bias rows at prep time.
  - L2's 64-feature B-half pairs 2 slots into one PSUM bank (partition
    offsets 0/64) -> exp once per pair; L3's 32-feature tail packs 4
    slots (offsets 0/32/64/96). w3 stationaries are parity-permuted so
    odd slots' kc1 features sit at partitions 64-127.
  - L3 sums ride the stt accum_out (fp32, pre-quantization); L4 + mean
    + final sum in fp64 on host.
"""

import numpy as np
import ml_dtypes

import concourse.bacc as bacc
import concourse.tile as tile
import concourse.mybir as mybir
from concourse.bass_utils import run_bass_kernel_spmd

F32 = mybir.dt.float32
BF16 = mybir.dt.bfloat16
F8 = mybir.dt.float8e4
DR = mybir.MatmulPerfMode.DoubleRow
AF = mybir.ActivationFunctionType
ALU = mybir.AluOpType

NP_F8 = ml_dtypes.float8_e4m3
NP_BF = ml_dtypes.bfloat16

S = 4
E = 8
N = 50000
AEV = 1008
ALPHA = 0.1
NCORES = 8
NA = N // S // 2           # atoms per core: 6250
CH = 512                   # atom chunk (matmul free dim)
NCH = (NA + CH - 1) // CH  # 13 chunks (12 x 512 + 106)
NSLOT = E * NCH            # 104 pipeline slots
NPK = NSLOT // 4           # 26 groups of 4 slots sharing the zv bank
S1 = 8.0
S2 = 16.0
S3 = 128.0
D1, D2, D3 = 256, 192, 160
NCORR = 8192               # atoms sampled for mean-correction

# exp consts: u = exp(z/(Ssc*a) + ln(Ssc*a) - 1) = Ssc*a*e^{y/a}
#   (z = Ssc*(y+a) includes bias)
EB1 = float(np.log(S1 * ALPHA) - 1.0)
EB2 = float(np.log(S2 * ALPHA) - 1.0)
EB3 = float(np.log(S3 * ALPHA) - 1.0)


def _build(dbg=False):
    nc = bacc.Bacc("TRN2", target_bir_lowering=False, debug=False,
                   num_devices=NCORES)

    x8 = nc.dram_tensor("x8", [4, 128, 2, NA], F8, kind="ExternalInput")
    w1 = nc.dram_tensor("w1", [E, 4, 128, 2, D1], F8, kind="ExternalInput")
    w2 = nc.dram_tensor("w2", [E, 128, 2, 256], F8, kind="ExternalInput")
    w2b = nc.dram_tensor("w2b", [E, 2, 128], BF16, kind="ExternalInput")
    w2bp = nc.dram_tensor("w2bp", [E // 2, 2, 128], BF16, kind="ExternalInput")
    w3 = nc.dram_tensor("w3", [E, 128, 2, 256], F8, kind="ExternalInput")
    g2i = nc.dram_tensor("g2i", [64, CH], F8, kind="ExternalInput")
    accA = nc.dram_tensor("accA", [128, NSLOT], F32, kind="ExternalOutput")
    accB = nc.dram_tensor("accB", [128, NPK], F32, kind="ExternalOutput")

    with tile.TileContext(nc) as tc:
        with (
            tc.tile_pool(name="wp", bufs=1) as wp,
            tc.tile_pool(name="xp", bufs=2) as xp,
            tc.tile_pool(name="g1p", bufs=4) as g1p,
            tc.tile_pool(name="g2p", bufs=6) as g2p,
            tc.tile_pool(name="up", bufs=3) as up,
            tc.tile_pool(name="sp", bufs=2) as sp,
            tc.tile_pool(name="ps", bufs=6, space="PSUM") as ps,
            tc.tile_pool(name="psb", bufs=1, space="PSUM") as psb,
            tc.tile_pool(name="psv", bufs=1, space="PSUM") as psv,
        ):
            # ---- x prefetch ----
            xtiles = {}

            def emit_x(ci):
                if ci >= NCH or ci in xtiles:
                    return
                off = ci * CH
                na = min(CH, NA - off)
                lst = []
                for p in range(4):
                    t = xp.tile([128, 2, CH], F8, tag=f"x{p}")
                    nc.sync.dma_start(t[:, :, :na], x8[p, :, :, off:off + na])
                    lst.append(t)
                xtiles[ci] = lst

            emit_x(0)
            emit_x(1)

            # ---- resident weights (e-major so e=0 lands first) ----
            w1t, w2t, w2bt, w2bpt, w3t = {}, {}, {}, {}, {}
            for e in range(E):
                for p in range(4):
                    t = wp.tile([128, 2, D1], F8, tag=f"w1_{e}_{p}")
                    nc.sync.dma_start(t[:], w1[e, p])
                    w1t[e, p] = t
                t = wp.tile([128, 2, 256], F8, tag=f"w2_{e}")
                nc.sync.dma_start(t[:], w2[e])
                w2t[e] = t
                t = wp.tile([2, 128], BF16, tag=f"w2b_{e}")
                nc.sync.dma_start(t[:], w2b[e])
                w2bt[e] = t
                if e % 2 == 0:
                    t = wp.tile([2, 128], BF16, tag=f"w2bp_{e // 2}")
                    nc.sync.dma_start(t[:], w2bp[e // 2])
                    w2bpt[e // 2] = t
                t = wp.tile([128, 2, 256], F8, tag=f"w3_{e}")
                nc.sync.dma_start(t[:], w3[e])
                w3t[e] = t
            accAt = wp.tile([128, NSLOT], F32, tag="accA")
            accBt = wp.tile([128, NPK], F32, tag="accB")
            ones2 = wp.tile([2, CH], BF16, tag="ones2")
            nc.vector.memset(ones2[:], 1.0)
            eb1c = wp.tile([128, 1], F32, tag="eb1c")
            nc.vector.memset(eb1c[:], EB1)
            eb2c = wp.tile([128, 1], F32, tag="eb2c")
            nc.vector.memset(eb2c[:], EB2)
            eb3c = wp.tile([128, 1], F32, tag="eb3c")
            nc.vector.memset(eb3c[:], EB3)

            state = {}
            pstate = {}   # zB pair banks, keyed it//2
            vstate = {}   # zv pack banks, keyed it//4

            def slot_info(it):
                ci, e = divmod(it, E)
                na = min(CH, NA - ci * CH)
                return ci, e, na

            def l1_mm(it):
                ci, e, na = slot_info(it)
                if it % E == 0:
                    emit_x(ci + 1)
                st = state.setdefault(it, {})
                st["z1"] = []
                for mi in range(2):
                    z = ps.tile([128, CH], F32, tag="z")
                    for p in range(4):
                        nc.tensor.matmul(
                            z[:, :na],
                            w1t[e, p][:, :, mi * 128:(mi + 1) * 128],
                            xtiles[ci][p][:, :, :na],
                            start=(p == 0), stop=(p == 3), perf_mode=DR,
                        )
                    st["z1"].append(z)

            def l1_ew(it):
                ci, e, na = slot_info(it)
                st = state[it]
                g1 = g1p.tile([128, 2, CH], F8, tag="g1")
                for mi in range(2):
                    z = st["z1"][mi]
                    u = up.tile([128, CH], BF16, tag=f"u1{mi}")
                    nc.scalar.activation(u[:, :na], z[:, :na], AF.Exp,
                                         bias=eb1c[:, 0:1], scale=float(10.0 / S1))
                    nc.vector.scalar_tensor_tensor(
                        g1[:, mi, :na], u[:, :na], float(S1 * ALPHA),
                        z[:, :na], op0=ALU.min, op1=ALU.max)
                st["g1"] = g1
                del st["z1"]

            def l2_mm(it):
                ci, e, na = slot_info(it)
                st = state[it]
                g1 = st["g1"]
                zA = ps.tile([128, CH], F32, tag="z")
                nc.tensor.matmul(zA[:, :na], w2bt[e][:, 0:128],
                                 ones2[:, :na], start=True, stop=False)
                nc.tensor.matmul(zA[:, :na], w2t[e][:, :, 0:128],
                                 g1[:, :, :na], start=False, stop=True,
                                 perf_mode=DR)
                k = it % 2
                if k == 0:
                    zb = psb.tile([128, CH], F32, tag="zbp", name="zbp")
                    pstate[it // 2] = zb
                    nc.tensor.matmul(zb[:, :na], w2bpt[e // 2][:, 0:128],
                                     ones2[:, :na], start=True, stop=False)
                zb = pstate[it // 2]
                if k == 0:
                    nc.tensor.matmul(zb[0:64, :na], w2t[e][:, :, 128:192],
                                     g1[:, :, :na], start=False, stop=False,
                                     perf_mode=DR)
                else:
                    # odd e: B features at weight cols 192-255 -> partitions
                    # 64-127; cols 128-191 are zero (accumulate +0 on even's)
                    nc.tensor.matmul(zb[:, :na], w2t[e][:, :, 128:256],
                                     g1[:, :, :na], start=False, stop=True,
                                     perf_mode=DR)
                st["zA"] = zA

            def l2_ew(it):
                ci, e, na = slot_info(it)
                st = state[it]
                zA = st["zA"]
                g2 = g2p.tile([128, 2, CH], F8, tag="g2")
                if it < 8:
                    # one-time per pool buf (bufs=4, parity-stable):
                    # bias const rows (1, 1/16, 1/256) + zeroed junk
                    # partitions for the L3 DR contraction (NaN*0 = NaN)
                    if it % 2 == 0:
                        nc.sync.dma_start(g2[64:128, 1, :], g2i[:])
                    else:
                        nc.sync.dma_start(g2[0:64, 1, :], g2i[:])
                uA = up.tile([128, CH], BF16, tag="u2A")
                nc.scalar.activation(uA[:, :na], zA[:, :na], AF.Exp,
                                     bias=eb2c[:, 0:1], scale=float(10.0 / S2))
                nc.vector.scalar_tensor_tensor(
                    g2[:, 0, :na], uA[:, :na], float(S2 * ALPHA),
                    zA[:, :na], op0=ALU.min, op1=ALU.max)
                st["g2"] = g2
                if it % 2 == 1:
                    zb = pstate.pop(it // 2)
                    uB = up.tile([128, CH], BF16, tag="u2B")
                    nc.scalar.activation(uB[:, :na], zb[:, :na], AF.Exp,
                                         bias=eb2c[:, 0:1], scale=float(10.0 / S2))
                    g2ev = state[it - 1]["g2"]
                    nc.vector.scalar_tensor_tensor(
                        g2ev[0:64, 1, :na], uB[0:64, :na], float(S2 * ALPHA),
                        zb[0:64, :na], op0=ALU.min, op1=ALU.max)
                    nc.vector.scalar_tensor_tensor(
                        g2[64:128, 1, :na], uB[64:128, :na], float(S2 * ALPHA),
                        zb[64:128, :na], op0=ALU.min, op1=ALU.max)

            def l3_mm(it):
                ci, e, na = slot_info(it)
                st = state[it]
                g2 = st["g2"]
                zA = ps.tile([128, CH], F32, tag="z")
                nc.tensor.matmul(zA[:, :na], w3t[e][:, :, 0:128],
                                 g2[:, :, :na], start=True, stop=True,
                                 perf_mode=DR)
                k = it % 4
                if k == 0:
                    vstate[it // 4] = psv.tile([128, CH], F32, tag="zvp",
                                               name="zvp")
                zv = vstate[it // 4]
                # tail features at weight cols 128+32k..128+32k+31; all other
                # cols zero -> full-M write at tile position (0,0)
                nc.tensor.matmul(zv[:, :na], w3t[e][:, :, 128:256],
                                 g2[:, :, :na], start=(k == 0), stop=(k == 3),
                                 perf_mode=DR)
                st["z3"] = zA

            def l3_ew(it):
                ci, e, na = slot_info(it)
                st = state[it]
                zA = st["z3"]
                u3 = up.tile([128, CH], BF16, tag="u3")
                nc.scalar.activation(u3[:, :na], zA[:, :na], AF.Exp,
                                     bias=eb3c[:, 0:1], scale=float(10.0 / S3))
                g3 = sp.tile([128, CH], BF16, tag="g3")
                nc.vector.scalar_tensor_tensor(
                    g3[:, :na], u3[:, :na], float(S3 * ALPHA), zA[:, :na],
                    op0=ALU.min, op1=ALU.max,
                    accum_out=accAt[:, it:it + 1])
                if it % 4 == 3:
                    zv = vstate.pop(it // 4)
                    uB = up.tile([128, CH], BF16, tag="u3B")
                    nc.scalar.activation(uB[:, :na], zv[:, :na], AF.Exp,
                                         bias=eb3c[:, 0:1], scale=float(10.0 / S3))
                    gB = sp.tile([128, CH], BF16, tag="g3B")
                    nc.vector.scalar_tensor_tensor(
                        gB[:, :na], uB[:, :na], float(S3 * ALPHA),
                        zv[:, :na], op0=ALU.min, op1=ALU.max,
                        accum_out=accBt[:, it // 4:it // 4 + 1])
                del state[it]

            # ---- software-pipelined main loop ----
            # stage offsets 3 apart so PE never waits on same-iteration
            # ACT/DVE g-tile writes; all matmuls emitted before elementwise
            for t in range(NSLOT + 6):
                if t < NSLOT:
                    l1_mm(t)
                if 3 <= t < NSLOT + 3:
                    l2_mm(t - 3)
                if t >= 6:
                    l3_mm(t - 6)
                if t < NSLOT:
                    l1_ew(t)
                if 3 <= t < NSLOT + 3:
                    l2_ew(t - 3)
                if t >= 6:
                    l3_ew(t - 6)

            nc.sync.dma_start(accA[:], accAt[:])
            nc.sync.dma_start(accB[:], accBt[:])
    nc.compile()
    return nc


_NC = None


def _get_nc():
    global _NC
    if _NC is None:
        _NC = _build()
    return _NC


def _f8(x):
    return np.clip(x, -240.0, 240.0).astype(NP_F8).astype(np.float32)


def _layer_host(z, Ssc):
    with np.errstate(over="ignore"):
        u = np.exp(z * np.float32(1.0 / (Ssc * ALPHA))
                   + np.float32(np.log(Ssc * ALPHA) - 1.0)
                   ).astype(NP_BF).astype(np.float32)
    return np.maximum(np.minimum(u, np.float32(Ssc * ALPHA)), z)


def _prep_inputs(inputs):
    aev = np.asarray(inputs["aev"], dtype=np.float32).reshape(N, AEV)
    idx = np.asarray(inputs["idx"])
    Ws = [np.asarray(inputs[f"W{i}"], dtype=np.float32) for i in (1, 2, 3, 4)]
    bs = [np.asarray(inputs[f"b{i}"], dtype=np.float32) for i in (1, 2, 3, 4)]

    in_maps = []
    per_species = {}
    for c in range(NCORES):
        s, h = c // 2, c % 2
        if s not in per_species:
            # ---- L1 fp8 stack with dual bias rows (as v4) ----
            w1s = np.zeros((E, 1024, D1), dtype=np.float32)
            w1s[:, :AEV, :] = Ws[0][s] * S1
            bt = (bs[0][s][:, 0, :] + ALPHA) * S1           # [E, 256]
            r1 = _f8(bt)
            r2 = _f8((bt - r1) * 16.0)
            w1s[:, AEV, :] = r1
            w1s[:, AEV + 1, :] = r2
            w1q = w1s.astype(NP_F8)
            w1pk = np.ascontiguousarray(
                w1q.reshape(E, 4, 2, 128, D1).transpose(0, 1, 3, 2, 4))

            # ---- sample atoms for mean-correction ----
            sel_s = np.asarray(idx[s])
            samp = aev[sel_s[:NCORR]]
            xq_s = _f8(samp)
            xaug = np.concatenate(
                [xq_s, np.ones((xq_s.shape[0], 1), np.float32),
                 np.full((xq_s.shape[0], 1), 1.0 / 16.0, np.float32)], axis=1)

            # ---- L2 fp8 DR stack + bias hi/lo + mean corr ----
            w2q_f = np.empty((E, D1, D2), dtype=np.float32)
            pre2 = np.empty((E, D2), dtype=np.float64)
            w3q_f = np.empty((E, 192, D3), dtype=np.float32)
            b3t = np.empty((E, D3), dtype=np.float64)
            for e in range(E):
                w2q = _f8(Ws[1][s, e] * (S2 / S1))           # [256, 192]
                w2q_f[e] = w2q
                b2p = bs[1][s, e, 0].astype(np.float64) - ALPHA * (
                    w2q.astype(np.float64) * (S1 / S2)).sum(axis=0)
                p2 = S2 * (b2p + ALPHA)
                # mean correction via sampled g1
                w1full = w1q[e, :AEV + 2].astype(np.float32)
                z1s = xaug @ w1full           # bias rows included via xaug
                g1s = _f8(_layer_host(z1s, S1))
                dW2 = w2q.astype(np.float64) - Ws[1][s, e].astype(np.float64) * (S2 / S1)
                Eg1 = g1s.astype(np.float64).mean(axis=0)
                p2 = p2 - dW2.T @ Eg1
                pre2[e] = p2

                w3q = _f8(Ws[2][s, e] * (S3 / S2))           # [192, 160]
                w3q_f[e] = w3q
                b3p = bs[2][s, e, 0].astype(np.float64) - ALPHA * (
                    w3q.astype(np.float64) * (S2 / S3)).sum(axis=0)
                bt3 = S3 * (b3p + ALPHA)
                z2s = g1s @ w2q + p2.astype(np.float32)
                g2s = _f8(_layer_host(z2s, S2))
                dW3 = w3q.astype(np.float64) - Ws[2][s, e].astype(np.float64) * (S3 / S2)
                Eg2 = g2s.astype(np.float64).mean(axis=0)
                b3t[e] = bt3 - dW3.T @ Eg2

            # pack L2 stationary [E, 128, 2, 256]: k = kc*128 + part;
            # cols 0-127 = A feats; B feats at 128-191 (even e) / 192-255
            # (odd e), zero elsewhere (-> partition-packed pair writes)
            w2pad = np.zeros((E, D1, 256), dtype=np.float32)
            w2pad[:, :, 0:128] = w2q_f[:, :, 0:128]
            for e in range(E):
                if e % 2 == 0:
                    w2pad[e, :, 128:192] = w2q_f[e, :, 128:192]
                else:
                    w2pad[e, :, 192:256] = w2q_f[e, :, 128:192]
            w2pk = np.ascontiguousarray(
                w2pad.astype(NP_F8).reshape(E, 2, 128, 256).transpose(0, 2, 1, 3))
            # L2 bias hi/lo rows
            p2hi = pre2.astype(np.float32).astype(NP_BF)
            p2lo = (pre2 - p2hi.astype(np.float64)).astype(np.float32).astype(NP_BF)
            w2bpk = np.zeros((E, 2, 128), dtype=NP_BF)
            w2bpk[:, 0, :] = p2hi[:, :128]
            w2bpk[:, 1, :] = p2lo[:, :128]
            w2bppk = np.zeros((E // 2, 2, 128), dtype=NP_BF)
            for pe in range(E // 2):
                w2bppk[pe, 0, 0:64] = p2hi[2 * pe, 128:192]
                w2bppk[pe, 1, 0:64] = p2lo[2 * pe, 128:192]
                w2bppk[pe, 0, 64:128] = p2hi[2 * pe + 1, 128:192]
                w2bppk[pe, 1, 64:128] = p2lo[2 * pe + 1, 128:192]

            # ---- L3 fp8 DR stack, parity kc1 layout + 3 fp8 bias rows ----
            # M cols: 0-127 = features 0-127; 128-255 = zv block where slot
            # k=e%4 owns cols 128+32k..128+32k+31 (features 128-159)
            b1r = _f8(b3t)
            b2r = _f8((b3t - b1r) * 16.0)
            b3r = _f8((b3t - b1r - b2r / 16.0) * 256.0)
            w3pk = np.zeros((E, 128, 2, 256), dtype=NP_F8)
            for e in range(E):
                wfull = np.zeros((192, 256), dtype=np.float32)  # [g2feat, M]
                brow = np.zeros((3, 256), dtype=np.float32)
                wfull[:, 0:128] = w3q_f[e][:, 0:128]
                brow[0, 0:128] = b1r[e, 0:128]
                brow[1, 0:128] = b2r[e, 0:128]
                brow[2, 0:128] = b3r[e, 0:128]
                k = e % 4
                c0 = 128 + 32 * k
                wfull[:, c0:c0 + 32] = w3q_f[e][:, 128:160]
                brow[0, c0:c0 + 32] = b1r[e, 128:160]
                brow[1, c0:c0 + 32] = b2r[e, 128:160]
                brow[2, c0:c0 + 32] = b3r[e, 128:160]
                w3pk[e, :, 0, :] = wfull[0:128].astype(NP_F8)
                if e % 2 == 0:
                    w3pk[e, 0:64, 1, :] = wfull[128:192].astype(NP_F8)
                    w3pk[e, 64:67, 1, :] = brow.astype(NP_F8)
                else:
                    w3pk[e, 64:128, 1, :] = wfull[128:192].astype(NP_F8)
                    w3pk[e, 0:3, 1, :] = brow.astype(NP_F8)

            per_species[s] = (w1pk, w2pk, w2bpk, w2bppk, w3pk)

        w1pk, w2pk, w2bpk, w2bppk, w3pk = per_species[s]
        sel = np.asarray(idx[s, h * NA:(h + 1) * NA])
        xTc = np.zeros((1024, NA), dtype=np.float32)
        xTc[:AEV] = aev[sel].T
        xTc[AEV] = 1.0
        xTc[AEV + 1] = 1.0 / 16.0
        x8c = np.ascontiguousarray(
            xTc.astype(NP_F8).reshape(4, 2, 128, NA).transpose(0, 2, 1, 3))

        g2init = np.zeros((64, CH), dtype=NP_F8)
        g2init[0, :] = 1.0
        g2init[1, :] = 1.0 / 16.0
        g2init[2, :] = 1.0 / 256.0
        in_maps.append({
            "x8": x8c, "w1": w1pk, "w2": w2pk, "w2b": w2bpk,
            "w2bp": w2bppk, "w3": w3pk, "g2i": g2init,
        })
    return in_maps, Ws, bs


def _finish(results, Ws, bs):
    W4 = Ws[3].astype(np.float64)  # [S, E, 160, 1]
    b4 = bs[3].astype(np.float64)  # [S, E, 1, 1]
    total = 0.0
    for c in range(NCORES):
        s = c // 2
        aA = results[c]["accA"].astype(np.float64)  # [128, NSLOT]
        aB = results[c]["accB"].astype(np.float64)  # [128, NPK]
        for e in range(E):
            g3sum = np.zeros(D3, dtype=np.float64)
            for ci in range(NCH):
                it = ci * E + e
                g3sum[:128] += aA[:, it]
                k = it % 4
                g3sum[128:160] += aB[32 * k:32 * k + 32, it // 4]
            h3sum = g3sum / S3 - ALPHA * NA
            total += (h3sum @ W4[s, e, :, 0] + NA * b4[s, e, 0, 0]) / E
    return np.array([total], dtype=np.float32)


def _run(inputs, **spmd_kwargs):
    in_maps, Ws, bs = _prep_inputs(inputs)
    nc = _get_nc()
    res = run_bass_kernel_spmd(nc, in_maps, list(range(NCORES)), **spmd_kwargs)
    return _finish(res.results, Ws, bs), res


def kernel(**inputs) -> np.ndarray:
    out, _ = _run(inputs)
    return out
